# revision 10
# baseline (speedup 1.0000x reference)
# Trainium2 Bass kernel for nn_AttentionLayer_69380901699611.
#
# Full-input contract: kernel(**inputs) takes the unsharded numpy inputs and
# returns the full [B, F, HIDDEN] output. Internally the work is sharded over
# 8 NeuronCores as (batch x head-group): core c handles batch c//4 and heads
# [4*(c%4), 4*(c%4)+4). Each core computes a partial output projection over
# its 4 heads; the host sums the 4 partials per batch.
#
# v2 design (ACT-bound):
#   - q/k/v projections and the score matmuls run in fp8-e4m3 with the
#     DoubleRow perf mode (0.5 PE cycles/row, 2 k-tiles per instruction).
#     Weights are scaled x32 on the host so e4m3 has mantissa to work with;
#     the scale is undone by the exp's scale immediate (2^-13 = 1/(32*32*8),
#     folding in the 1/sqrt(depth) factor) and by wo/32.
#   - For scores, head h's 64-deep contraction is split into 2 k-tiles of 32
#     held at SBUF partitions 32h..32h+31 (host reorders weight columns into
#     A=d0-31 / B=d32-63 blocks), so one DR matmul per (head, T-tile) emits
#     [T=128, F=512] scores.
#   - softmax: exp on ACT (PSUM->SBUF bf16, scale immediate), then *exp(bias)
#     with DVE (heads 0-2, one 2x-mode instr) + GPSIMD (head 3) so DVE stays
#     under the ACT roofline. No max-subtraction needed: |logits/8| <~ 12.
#   - attention accumulates in [F-part 128, head, 65] layout (full-partition
#     matmuls with pt as stationary, v+ones as moving; col 64 = softmax
#     denominator), one PSUM bank per 128-F tile. Normalize is then a tiny
#     per-F-tile reciprocal+mult, PE-transposed into attnT for the bf16
#     output projection.
#   - ACT runs 128 exp instructions of N=1024 back-to-back (~133us); all
#     other engines are strictly below that.

import numpy as np

B, F, T, C = 2, 2048, 2048, 1024
HEADS, DEPTH = 16, 64
N_CORES = 8
HG = 4   # head-groups; heads per group = HEADS // HG = 4
WSC = 32.0  # host-side weight scale for e4m3


def build_attention_nc(C=1024, F=2048, T=2048, NHEADS=4, H=64, fc_w=512):
    import concourse.tile as tile
    import concourse.mybir as mybir
    from concourse import bacc

    P = 128
    NH = NHEADS * H          # 256
    KC = C // P              # 8 k-tiles for the projections
    NFC = F // fc_w          # 4 F chunks
    NTT = T // P             # 16 T tiles
    FPC = fc_w // P          # 4 F tiles per chunk
    f32 = mybir.dt.float32
    bf16 = mybir.dt.bfloat16
    f8e4 = mybir.dt.float8e4
    Exp = mybir.ActivationFunctionType.Exp
    Mult = mybir.AluOpType.mult
    DR = mybir.MatmulPerfMode.DoubleRow
    ESCALE = 1.0 / (WSC * WSC * H ** 0.5)  # 2^-13

    nc = bacc.Bacc("TRN2", target_bir_lowering=False, debug=False, name="attn69")

    qT_d = nc.dram_tensor("qT", [C, F], bf16, kind="ExternalInput")
    sT_d = nc.dram_tensor("sT", [C, T], bf16, kind="ExternalInput")
    eb_d = nc.dram_tensor("ebT", [T, F], bf16, kind="ExternalInput")
    # wq/wk columns: [A: h0 d0-31 | h1 d0-31 | ... | B: h0 d32-63 | ...]
    wq_d = nc.dram_tensor("wq", [C, 2, P], bf16, kind="ExternalInput")
    wk_d = nc.dram_tensor("wk", [C, 2, P], bf16, kind="ExternalInput")
    wv_d = nc.dram_tensor("wv", [C, NH], bf16, kind="ExternalInput")
    wo_d = nc.dram_tensor("wo", [NH, C], bf16, kind="ExternalInput")
    id_d = nc.dram_tensor("ident", [P, P], f32, kind="ExternalInput")
    out_d = nc.dram_tensor("out_p", [F, C], f32, kind="ExternalOutput")

    with tile.TileContext(nc) as tc:
        with (
            tc.tile_pool(name="constp", bufs=1) as constp,
            tc.tile_pool(name="persist", bufs=1) as persist,
            tc.tile_pool(name="qap", bufs=2) as qap,
            tc.tile_pool(name="sap", bufs=4) as sap,
            tc.tile_pool(name="biasp", bufs=6) as biasp,
            tc.tile_pool(name="ptp", bufs=18) as ptp,
            tc.tile_pool(name="flatp", bufs=2) as flatp,
            tc.tile_pool(name="smallp", bufs=4) as smallp,
            tc.tile_pool(name="outp", bufs=6) as outp,
            tc.tile_pool(name="psA", bufs=4, space="PSUM") as psA,
            tc.tile_pool(name="psS", bufs=2, space="PSUM") as psS,
        ):
            # weights on the critical path first; wv/wo/ident deferred
            wq_sb = constp.tile([P, KC, 2, P], bf16, name="wq_sb")
            nc.sync.dma_start(wq_sb[:], wq_d.ap().rearrange("(ko p) a m -> p ko a m", p=P))
            wk_sb = constp.tile([P, KC, 2, P], bf16, name="wk_sb")
            nc.sync.dma_start(wk_sb[:], wk_d.ap().rearrange("(ko p) a m -> p ko a m", p=P))
            wv_sb = constp.tile([P, KC, NH], bf16, name="wv_sb")
            wo_sb = constp.tile([P, 2, C], bf16, name="wo_sb")
            ident = constp.tile([P, P], f32, name="ident")

            # ---------------- persistent activations ----------------
            # qT/kT: [4 heads x 32 depth on partitions, A/B k-tile, cols]
            qT_sb = persist.tile([P, 2, F], f8e4, name="qT_sb")
            kT_sb = persist.tile([P, 2, T], f8e4, name="kT_sb")
            v_sb = persist.tile([P, NTT, NHEADS, H + 1], bf16, name="v_sb")
            attnT_sb = persist.tile([P, 2, F], bf16, name="attnT_sb")
            # ones column for the softmax denominator
            nc.vector.memset(v_sb[:, :, :, H:H + 1], 1.0)

            qT_r = qT_d.ap().rearrange("(ko p) f -> p ko f", p=P)
            sT_r = sT_d.ap().rearrange("(ko p) t -> p ko t", p=P)
            sa_tiles = {}

            def q_proj(fc):
                fsl = slice(fc * fc_w, (fc + 1) * fc_w)
                qa = qap.tile([P, KC, fc_w], bf16, tag="qa", name="qa")
                nc.sync.dma_start(qa[:], qT_r[:, :, fsl])
                for a in range(2):
                    psq = psA.tile([P, 512], f32, tag="bank", name="psq")
                    for k in range(KC):
                        nc.tensor.matmul(
                            psq[:, :fc_w],
                            lhsT=wq_sb[:, k, a, :],
                            rhs=qa[:, k, :],
                            start=(k == 0), stop=(k == KC - 1))
                    nc.vector.tensor_copy(qT_sb[:, a, fsl], psq[:, :fc_w])

            def k_proj(sc, csl=slice(0, fc_w)):
                # csl: column subrange of the chunk (for first-tile splitting)
                ssl = slice(sc * fc_w + csl.start, sc * fc_w + csl.stop)
                sa = sa_tiles[sc]
                n = csl.stop - csl.start
                for a in range(2):
                    psk = psA.tile([P, 512], f32, tag="bank", name="psk")
                    for k in range(KC):
                        nc.tensor.matmul(
                            psk[:, :n],
                            lhsT=wk_sb[:, k, a, :],
                            rhs=sa[:, k, csl],
                            start=(k == 0), stop=(k == KC - 1))
                    nc.vector.tensor_copy(kT_sb[:, a, ssl], psk[:, :n])

            def v_piece(tt):
                sc, tl = tt // (fc_w // P), tt % (fc_w // P)
                sa = sa_tiles[sc]
                psv = psA.tile([P, 512], f32, tag="bank", name="psv")
                for k in range(KC):
                    nc.tensor.matmul(
                        psv[:, :NH],
                        lhsT=sa[:, k, tl * P:(tl + 1) * P],
                        rhs=wv_sb[:, k, :],
                        start=(k == 0), stop=(k == KC - 1))
                nc.vector.tensor_copy(
                    v_sb[:, tt, :, 0:H],
                    psv[:, :NH].rearrange("p (h x) -> p h x", h=NHEADS))

            # ------------- softmax stream -------------
            pt_store = {}

            def produce(fc, tt):
                fsl = slice(fc * fc_w, (fc + 1) * fc_w)
                tsl = slice(tt * P, (tt + 1) * P)
                bias_t = biasp.tile([P, fc_w], bf16, tag="bias", name="bias_t")
                nc.sync.dma_start(bias_t[:], eb_d.ap()[tsl, fsl])
                pt4 = ptp.tile([P, NHEADS, fc_w], bf16, tag="pt", name="pt4")
                for pair in range(2):
                    st2 = psS.tile([P, 2, 512], f32, tag="st", name="st2")
                    for j in range(2):
                        h = 2 * pair + j
                        nc.tensor.matmul(
                            st2[:, j, :fc_w],
                            lhsT=kT_sb[32 * h:32 * h + 32, :, tsl],
                            rhs=qT_sb[32 * h:32 * h + 32, :, fsl],
                            start=True, stop=True,
                            perf_mode=DR, tile_position=(32 * h, 0))
                    nc.scalar.activation(
                        pt4[:, 2 * pair:2 * pair + 2, :], st2[:, :, :fc_w],
                        Exp, scale=ESCALE)
                # *exp(bias): heads 0-2 on DVE (2x mode), head 3 on GPSIMD
                nc.vector.tensor_mul(
                    pt4[:, 0:3, :], pt4[:, 0:3, :],
                    bias_t[:, None, :].to_broadcast((P, 3, fc_w)))
                nc.gpsimd.tensor_tensor(
                    pt4[:, 3, :], pt4[:, 3, :], bias_t[:], Mult)
                pt_store[(fc, tt)] = pt4

            def alloc_at():
                ats = []
                for fl in range(FPC):
                    raw = psA.tile([P, 512], f32, tag="bank", name=f"at{fl}")
                    ats.append(raw[:, :NHEADS * (H + 1)].rearrange(
                        "p (h x) -> p h x", h=NHEADS))
                return ats

            def consume(fc, tt, ats):
                # One PSUM accumulation group per bank: start=True lazily
                # zeroes the WHOLE 2KB zero region, so only the first write
                # into each F-tile bank may carry it; only the last carries
                # stop (sim-only bookkeeping).
                pt4 = pt_store.pop((fc, tt))
                for fl in range(FPC):
                    for h in range(NHEADS):
                        nc.tensor.matmul(
                            ats[fl][:, h, :],
                            lhsT=pt4[:, h, fl * P:(fl + 1) * P],
                            rhs=v_sb[:, tt, h, :],
                            start=(tt == 0 and h == 0),
                            stop=(tt == NTT - 1 and h == NHEADS - 1))

            flats = {}

            def fin_normalize(fc, ats):
                flat = flatp.tile([P, FPC, NHEADS, H], f32, tag="flat", name="flat")
                flats[fc] = flat
                for fl in range(FPC):
                    rec = smallp.tile([P, NHEADS, 1], f32, tag="rec", name="rec")
                    nc.vector.reciprocal(rec[:], ats[fl][:, :, H:H + 1])
                    nc.vector.tensor_tensor(
                        flat[:, fl, :, :], ats[fl][:, :, 0:H],
                        rec.to_broadcast((P, NHEADS, H)), Mult)

            def fin_transposes(fc):
                # both m-chunks' transposes into the two banks of one psS tile
                fsl = slice(fc * fc_w, (fc + 1) * fc_w)
                flat = flats.pop(fc)
                tr = psS.tile([P, 2, 512], f32, tag="st", name="tr")
                for m in range(2):
                    for fl in range(FPC):
                        nc.tensor.transpose(
                            tr[:, m, fl * P:(fl + 1) * P],
                            flat[:, fl, 2 * m:2 * m + 2, :], ident[:])
                    nc.vector.tensor_copy(attnT_sb[:, m, fsl], tr[:, m, :])

            def fin_outproj(fc, fl):
                # one F-tile's output projection (both 512-col halves)
                ft = fc * FPC + fl
                pso = psS.tile([P, 2, 512], f32, tag="st", name="pso")
                for cc in range(2):
                    for m in range(2):
                        nc.tensor.matmul(
                            pso[:, cc, :],
                            lhsT=attnT_sb[:, m, ft * P:(ft + 1) * P],
                            rhs=wo_sb[:, m, cc * 512:(cc + 1) * 512],
                            start=(m == 0), stop=(m == 1))
                    ot = outp.tile([P, 512], f32, tag="o", name="ot")
                    nc.vector.tensor_copy(ot[:], pso[:, cc, :])
                    nc.sync.dma_start(
                        out_d.ap()[ft * P:(ft + 1) * P, cc * 512:(cc + 1) * 512],
                        ot[:])

            # ------------- schedule -------------
            # Prefix: get the first exp running ASAP, then keep ACT saturated.
            # v-projection pieces are deferred into ACT-busy slots; wv/wo/ident
            # DMAs sit behind the first produce's dependencies.
            q_proj(0)
            sa0 = sap.tile([P, KC, fc_w], bf16, tag="sa", name="sa")
            nc.sync.dma_start(sa0[:], sT_r[:, :, 0:fc_w])
            sa_tiles[0] = sa0
            k_proj(0, slice(0, P))          # just the first T-tile's columns
            produce(0, 0)
            nc.sync.dma_start(wv_sb[:], wv_d.ap().rearrange("(ko p) m -> p ko m", p=P))
            nc.sync.dma_start(wo_sb[:], wo_d.ap().rearrange("(ko p) m -> p ko m", p=P))
            nc.sync.dma_start(ident[:], id_d.ap())
            k_proj(0, slice(P, fc_w))       # rest of chunk 0
            for tt in range(1, 4):
                produce(0, tt)
                v_piece(tt - 1)
            for sc in range(1, T // fc_w):
                sa = sap.tile([P, KC, fc_w], bf16, tag="sa", name="sa")
                nc.sync.dma_start(sa[:], sT_r[:, :, sc * fc_w:(sc + 1) * fc_w])
                sa_tiles[sc] = sa
                k_proj(sc)
                for tl in range(fc_w // P):
                    tt = sc * (fc_w // P) + tl
                    produce(0, tt)
                    v_piece(tt - 1)
            for fc in range(1, NFC):
                q_proj(fc)
            v_piece(NTT - 1)

            # Main loop: consume chunk fc-1 while producing chunk fc;
            # finish(fc-2)'s PE work is spread across early tt slots so the
            # score matmul stream (and thus ACT) never stalls at boundaries.
            at_store = {0: alloc_at()}
            for fc in range(1, NFC + 1):
                ats = at_store[fc - 1]
                for tt in range(NTT):
                    if fc < NFC:
                        produce(fc, tt)
                    if fc >= 2:
                        if tt == 0:
                            fin_transposes(fc - 2)
                        elif 1 <= tt <= FPC:
                            fin_outproj(fc - 2, tt - 1)
                    consume(fc - 1, tt, ats)
                fin_normalize(fc - 1, at_store.pop(fc - 1))
                if fc < NFC:
                    at_store[fc] = alloc_at()
            # tail: last chunk's transposes + output projection
            fin_transposes(NFC - 1)
            for fl in range(FPC):
                fin_outproj(NFC - 1, fl)

    nc.compile()
    return nc


_CACHE = {}


def _get_nc():
    if "nc" not in _CACHE:
        _CACHE["nc"] = build_attention_nc(C=C, F=F, T=T, NHEADS=HEADS // HG, H=DEPTH)
    return _CACHE["nc"]


def kernel(query_input, source_input, bias, wq, wk, wv, wo, **run_kwargs):
    import ml_dtypes
    from concourse.bass_utils import run_bass_kernel_spmd

    bf = ml_dtypes.bfloat16
    e4 = ml_dtypes.float8_e4m3
    q = np.asarray(query_input, dtype=np.float32)
    s = np.asarray(source_input, dtype=np.float32)
    b = np.asarray(bias, dtype=np.float32)
    wq4 = np.asarray(wq, dtype=np.float32) * WSC   # [C, 16, 64]
    wk4 = np.asarray(wk, dtype=np.float32) * WSC
    wv4 = np.asarray(wv, dtype=np.float32) * WSC
    wo4 = np.asarray(wo, dtype=np.float32) / WSC   # [16, 64, C]

    qT = [np.ascontiguousarray(q[i].T).astype(bf) for i in range(B)]
    sT = [np.ascontiguousarray(s[i].T).astype(bf) for i in range(B)]
    ebT = np.exp(np.ascontiguousarray(b[0, 0].T)).astype(bf)
    ident = np.eye(128, dtype=np.float32)

    nhl = HG  # heads per group
    in_maps = []
    for c in range(N_CORES):
        bi, hg = c // HG, c % HG
        hsl = slice(hg * nhl, (hg + 1) * nhl)
        # wq/wk: [C, 2(A|B), 4 heads, 32 depth] -> [C, 2, 128]
        wqg = wq4[:, hsl, :].reshape(C, HG, 2, 32).transpose(0, 2, 1, 3)
        wkg = wk4[:, hsl, :].reshape(C, HG, 2, 32).transpose(0, 2, 1, 3)
        in_maps.append({
            "qT": qT[bi],
            "sT": sT[bi],
            "ebT": ebT,
            "wq": np.ascontiguousarray(wqg.reshape(C, 2, 128)).astype(bf),
            "wk": np.ascontiguousarray(wkg.reshape(C, 2, 128)).astype(bf),
            "wv": np.ascontiguousarray(
                wv4[:, hsl, :].reshape(C, HG * DEPTH)).astype(bf),
            "wo": np.ascontiguousarray(
                wo4[hsl, :, :].reshape(HG * DEPTH, C)).astype(bf),
            "ident": ident,
        })

    nc = _get_nc()
    res = run_bass_kernel_spmd(nc, in_maps, core_ids=list(range(N_CORES)), **run_kwargs)
    _CACHE["last_results"] = res

    out = np.empty((B, F, C), np.float32)
    for bi in range(B):
        acc = res.results[bi * HG]["out_p"].astype(np.float32)
        for hg in range(1, HG):
            acc = acc + res.results[bi * HG + hg]["out_p"]
        out[bi] = acc
    return out


# revision 28
# speedup vs baseline: 1.1759x; 1.1759x over previous
# Trainium2 Bass kernel for nn_AttentionLayer_69380901699611.
#
# Full-input contract: kernel(**inputs) takes the unsharded numpy inputs and
# returns the full [B, F, HIDDEN] output. Internally the work is sharded over
# 8 NeuronCores as (batch x head-group): core c handles batch c//4 and heads
# [4*(c%4), 4*(c%4)+4). Each core computes a partial output projection over
# its 4 heads; the host sums the 4 partials per batch.
#
# v3 design — ACT (exp) is the roofline (128 exps of N=1024 at ~1038ns =
# ~133us); every other engine is scheduled to stay strictly below it:
#   - q/k/v projections bf16 (PE has slack; fp8 here costs accuracy).
#   - scores: fp8-e4m3 DoubleRow, one matmul per (head, T-tile): head h's
#     64-deep contraction is 2 k-tiles of 32 at partitions 32h..32h+31
#     (host pre-orders weight columns into A=d0-31 / B=d32-63 blocks).
#     Weights are scaled x32 so e4m3 has mantissa; the exp's scale immediate
#     2^-13 = 1/(32*32*sqrt(64)) undoes it (wo is divided by 32).
#   - softmax: exp on ACT (PSUM->SBUF bf16), *exp(bias) split DVE (heads
#     0-2, 2x mode) + GPSIMD (head 3). No max-subtraction: |logits/8| < ~12.
#   - attention accumulates in [F-part 128, head, 64] layout: 4 F-tiles in
#     2 PSUM banks (2 per bank); denominators via N=1 matmuls (pt.T @ ones)
#     into a shared bank. PSUM "start" bumps a per-bank epoch (stale-tag
#     writes overwrite), so each bank gets exactly ONE start/stop per chunk.
#   - normalize = reciprocal + one mult per F-tile; PE-transpose into attnT
#     via a dedicated finish bank; bf16 output projection, DMA'd per F-tile.
#   - PSUM: scores 2x2 banks + value-accum 2 + denom 1 + finish 1 = 8.
#   - Schedule: first-chunk DMAs split per k-tile so PE starts at ~2us;
#     v-projection pieces and late q-projections are spread into per-tt
#     slots (budget <= ~1.7us under the 2.08us/tt ACT pace); chunk 2's
#     consume is compressed 2-per-slot so chunk 3's consume starts
#     mid-iteration and the tail stays short.

import numpy as np

B, F, T, C = 2, 2048, 2048, 1024
HEADS, DEPTH = 16, 64
N_CORES = 8
HG = 4   # head-groups; heads per group = HEADS // HG = 4
WSC = 32.0  # host-side weight scale for e4m3


def build_attention_nc(C=1024, F=2048, T=2048, NHEADS=4, H=64, fc_w=512):
    import concourse.tile as tile
    import concourse.mybir as mybir
    from concourse import bacc

    P = 128
    NH = NHEADS * H          # 256
    KC = C // P              # 8 k-tiles for the projections
    NFC = F // fc_w          # 4 F chunks
    NTT = T // P             # 16 T tiles
    FPC = fc_w // P          # 4 F tiles per chunk
    f32 = mybir.dt.float32
    bf16 = mybir.dt.bfloat16
    f8e4 = mybir.dt.float8e4
    Exp = mybir.ActivationFunctionType.Exp
    Mult = mybir.AluOpType.mult
    DR = mybir.MatmulPerfMode.DoubleRow
    ESCALE = 1.0 / (WSC * WSC * H ** 0.5)  # 2^-13

    nc = bacc.Bacc("TRN2", target_bir_lowering=False, debug=False, name="attn69")

    qT_d = nc.dram_tensor("qT", [C, F], bf16, kind="ExternalInput")
    sT_d = nc.dram_tensor("sT", [C, T], bf16, kind="ExternalInput")
    eb_d = nc.dram_tensor("ebT", [T, F], bf16, kind="ExternalInput")
    # wq/wk columns: [A: h0 d0-31 | h1 d0-31 | ... | B: h0 d32-63 | ...]
    wq_d = nc.dram_tensor("wq", [C, 2, P], bf16, kind="ExternalInput")
    wk_d = nc.dram_tensor("wk", [C, 2, P], bf16, kind="ExternalInput")
    wv_d = nc.dram_tensor("wv", [C, NH], bf16, kind="ExternalInput")
    wo_d = nc.dram_tensor("wo", [NH, C], bf16, kind="ExternalInput")
    id_d = nc.dram_tensor("ident", [P, P], f32, kind="ExternalInput")
    out_d = nc.dram_tensor("out_p", [F, C], f32, kind="ExternalOutput")

    with tile.TileContext(nc) as tc:
        with (
            tc.tile_pool(name="constp", bufs=1) as constp,
            tc.tile_pool(name="persist", bufs=1) as persist,
            tc.tile_pool(name="qap", bufs=2) as qap,
            tc.tile_pool(name="sap", bufs=4) as sap,
            tc.tile_pool(name="biasp", bufs=6) as biasp,
            tc.tile_pool(name="ptp", bufs=18) as ptp,
            tc.tile_pool(name="flatp", bufs=2) as flatp,
            tc.tile_pool(name="smallp", bufs=4) as smallp,
            tc.tile_pool(name="outp", bufs=6) as outp,
            tc.tile_pool(name="psS", bufs=2, space="PSUM") as psS,     # 4 banks
            tc.tile_pool(name="vaccp", bufs=2, space="PSUM") as vaccp,  # 2 banks
            tc.tile_pool(name="denp", bufs=1, space="PSUM") as denp,    # 1 bank
            tc.tile_pool(name="finp", bufs=1, space="PSUM") as finp,    # 1 bank
        ):
            # weight tiles; only wq/wk DMA'd up front (split per k-tile),
            # the rest deferred off the critical path
            wq_sb = constp.tile([P, KC, 2, P], bf16, name="wq_sb")
            wk_sb = constp.tile([P, KC, 2, P], bf16, name="wk_sb")
            wv_sb = constp.tile([P, KC, NH], bf16, name="wv_sb")
            wo_sb = constp.tile([P, 2, C], bf16, name="wo_sb")
            ident = constp.tile([P, P], f32, name="ident")
            ones1 = constp.tile([P, 1], bf16, name="ones1")
            wq_r = wq_d.ap().rearrange("(ko p) a m -> p ko a m", p=P)
            wk_r = wk_d.ap().rearrange("(ko p) a m -> p ko a m", p=P)

            # persistent activations
            # qT/kT: [4 heads x 32 depth on partitions, A/B k-tile, cols]
            qT_sb = persist.tile([P, 2, F], f8e4, name="qT_sb")
            kT_sb = persist.tile([P, 2, T], f8e4, name="kT_sb")
            v_sb = persist.tile([P, NTT, NHEADS, H], bf16, name="v_sb")
            attnT_sb = persist.tile([P, 2, F], bf16, name="attnT_sb")
            nc.vector.memset(ones1[:], 1.0)

            qT_r = qT_d.ap().rearrange("(ko p) f -> p ko f", p=P)
            sT_r = sT_d.ap().rearrange("(ko p) t -> p ko t", p=P)
            sa_tiles = {}
            qa_tiles = {}
            qps = {}

            # ---- projections (piecewise emission) ----
            def q_dma(fc, ks=None):
                fsl = slice(fc * fc_w, (fc + 1) * fc_w)
                qa = qap.tile([P, KC, fc_w], bf16, tag="qa", name="qa")
                qa_tiles[fc] = qa
                if ks is None:
                    nc.sync.dma_start(qa[:], qT_r[:, :, fsl])
                else:
                    for k in ks:
                        nc.sync.dma_start(qa[:, k, :], qT_r[:, k, fsl])

            def q_piece(fc, a, half, pool):
                # half 0: k=0..3 (allocates psq), half 1: k=4..7 + copy
                fsl = slice(fc * fc_w, (fc + 1) * fc_w)
                qa = qa_tiles[fc]
                if half == 0:
                    qps[(fc, a)] = pool.tile([P, 512], f32, tag="bank", name="psq")
                psq = qps[(fc, a)]
                for k in range(4 * half, 4 * half + 4):
                    nc.tensor.matmul(
                        psq[:, :fc_w], lhsT=wq_sb[:, k, a, :], rhs=qa[:, k, :],
                        start=(k == 0), stop=(k == KC - 1))
                if half == 1:
                    nc.vector.tensor_copy(qT_sb[:, a, fsl], psq[:, :fc_w])
                    del qps[(fc, a)]

            def k_proj(sc, pool, csl=slice(0, fc_w)):
                ssl = slice(sc * fc_w + csl.start, sc * fc_w + csl.stop)
                sa = sa_tiles[sc]
                n = csl.stop - csl.start
                for a in range(2):
                    psk = pool.tile([P, 512], f32, tag="bank", name="psk")
                    for k in range(KC):
                        nc.tensor.matmul(
                            psk[:, :n], lhsT=wk_sb[:, k, a, :], rhs=sa[:, k, csl],
                            start=(k == 0), stop=(k == KC - 1))
                    nc.vector.tensor_copy(kT_sb[:, a, ssl], psk[:, :n])

            def v_piece(tt, pool):
                sc, tl = tt // FPC, tt % FPC
                sa = sa_tiles[sc]
                psv = pool.tile([P, 512], f32, tag="bank", name="psv")
                for k in range(KC):
                    nc.tensor.matmul(
                        psv[:, :NH], lhsT=sa[:, k, tl * P:(tl + 1) * P],
                        rhs=wv_sb[:, k, :],
                        start=(k == 0), stop=(k == KC - 1))
                nc.vector.tensor_copy(
                    v_sb[:, tt, :, :],
                    psv[:, :NH].rearrange("p (h x) -> p h x", h=NHEADS))

            # ---- softmax stream ----
            pt_store = {}

            mul_args = {}

            def produce(fc, tt, mul=True):
                fsl = slice(fc * fc_w, (fc + 1) * fc_w)
                tsl = slice(tt * P, (tt + 1) * P)
                bias_t = biasp.tile([P, fc_w], bf16, tag="bias", name="bias_t")
                nc.sync.dma_start(bias_t[:], eb_d.ap()[tsl, fsl])
                pt4 = ptp.tile([P, NHEADS, fc_w], bf16, tag="pt", name="pt4")
                for pair in range(2):
                    st2 = psS.tile([P, 2, 512], f32, tag="st", name="st2")
                    for j in range(2):
                        h = 2 * pair + j
                        nc.tensor.matmul(
                            st2[:, j, :fc_w],
                            lhsT=kT_sb[32 * h:32 * h + 32, :, tsl],
                            rhs=qT_sb[32 * h:32 * h + 32, :, fsl],
                            start=True, stop=True,
                            perf_mode=DR, tile_position=(32 * h, 0))
                    nc.scalar.activation(
                        pt4[:, 2 * pair:2 * pair + 2, :], st2[:, :, :fc_w],
                        Exp, scale=ESCALE)
                pt_store[(fc, tt)] = pt4
                mul_args[(fc, tt)] = (pt4, bias_t)
                if mul:
                    produce_mul(fc, tt)

            def produce_mul(fc, tt, dve_only=False):
                # *exp(bias). Normally heads 0-2 on DVE (2x mode) + head 3 on
                # GPSIMD; dve_only puts all 4 heads in one DVE instruction
                # (used for chunk 0 so its deferred muls don't flood Pool).
                pt4, bias_t = mul_args.pop((fc, tt))
                if dve_only:
                    nc.vector.tensor_mul(
                        pt4[:], pt4[:],
                        bias_t[:, None, :].to_broadcast((P, NHEADS, fc_w)))
                else:
                    nc.vector.tensor_mul(
                        pt4[:, 0:3, :], pt4[:, 0:3, :],
                        bias_t[:, None, :].to_broadcast((P, 3, fc_w)))
                    nc.gpsimd.tensor_tensor(
                        pt4[:, 3, :], pt4[:, 3, :], bias_t[:], Mult)

            # ---- attention accumulate ----
            acc = {}

            def alloc_acc(fc):
                vb = []
                for b in range(2):
                    raw = vaccp.tile([P, 512], f32, tag="bank", name=f"vacc{b}")
                    vb.append(raw.rearrange("p (r h x) -> p r h x", r=2, h=NHEADS))
                dn_raw = denp.tile([P, 512], f32, tag="den", name="dn")
                dn = dn_raw[:, :FPC * NHEADS].rearrange(
                    "p (fl h) -> p fl h", fl=FPC)
                acc[fc] = (vb, dn)

            def consume(fc, tt):
                # One PSUM epoch per bank and chunk: only the first write
                # carries start (bumps the bank epoch; stale-tag writes then
                # overwrite), only the last carries stop.
                pt4 = pt_store.pop((fc, tt))
                vb, dn = acc[fc]
                for fl in range(FPC):
                    b, r = fl // 2, fl % 2
                    lhs = pt4[:, :, fl * P:(fl + 1) * P]
                    for h in range(NHEADS):
                        nc.tensor.matmul(
                            vb[b][:, r, h, :], lhsT=lhs[:, h, :],
                            rhs=v_sb[:, tt, h, :],
                            start=(tt == 0 and r == 0 and h == 0),
                            stop=(tt == NTT - 1 and r == 1 and h == NHEADS - 1))
                        nc.tensor.matmul(
                            dn[:, fl, h:h + 1], lhsT=lhs[:, h, :],
                            rhs=ones1[:],
                            start=(tt == 0 and fl == 0 and h == 0),
                            stop=(tt == NTT - 1 and fl == FPC - 1
                                  and h == NHEADS - 1))

            # ---- finish (normalize / transpose / output projection) ----
            flats = {}

            def fin_normalize(fc):
                vb, dn = acc.pop(fc)
                flat = flatp.tile([P, FPC, NHEADS, H], f32, tag="flat", name="flat")
                flats[fc] = flat
                rec = smallp.tile([P, FPC, NHEADS, 1], f32, tag="rec", name="rec")
                nc.vector.reciprocal(rec[:], dn[:, :, :, None])
                for b in range(2):
                    nc.vector.tensor_tensor(
                        flat[:, 2 * b:2 * b + 2, :, :], vb[b][:],
                        rec[:, 2 * b:2 * b + 2, :, :].to_broadcast(
                            (P, 2, NHEADS, H)), Mult)

            def fin_tr(fc, m, pool):
                fsl = slice(fc * fc_w, (fc + 1) * fc_w)
                flat = flats[fc]
                tr = pool.tile([P, 512], f32, tag="bank", name="tr")
                for fl in range(FPC):
                    nc.tensor.transpose(
                        tr[:, fl * P:(fl + 1) * P],
                        flat[:, fl, 2 * m:2 * m + 2, :], ident[:])
                nc.vector.tensor_copy(attnT_sb[:, m, fsl], tr[:])
                if m == 1:
                    del flats[fc]

            def fin_pso(fc, piece, pool):
                # piece = (fl, cc): one 128F x 512C output block
                fl, cc = piece // 2, piece % 2
                ft = fc * FPC + fl
                pso = pool.tile([P, 512], f32, tag="bank", name="pso")
                for m in range(2):
                    nc.tensor.matmul(
                        pso[:], lhsT=attnT_sb[:, m, ft * P:(ft + 1) * P],
                        rhs=wo_sb[:, m, cc * 512:(cc + 1) * 512],
                        start=(m == 0), stop=(m == 1))
                ot = outp.tile([P, 512], f32, tag="o", name="ot")
                nc.vector.tensor_copy(ot[:], pso[:])
                nc.sync.dma_start(
                    out_d.ap()[ft * P:(ft + 1) * P, cc * 512:(cc + 1) * 512],
                    ot[:])

            # ================= schedule =================
            # Prefix: split first-chunk DMAs per k-tile so PE starts ~2us in;
            # first exp as soon as q-chunk 0 + first k T-tile are projected.
            qa0 = qap.tile([P, KC, fc_w], bf16, tag="qa", name="qa")
            qa_tiles[0] = qa0
            sa0 = sap.tile([P, KC, fc_w], bf16, tag="sa", name="sa")
            sa_tiles[0] = sa0
            # halved transfers ordered so the q matmuls start ~3us in; the
            # first k projection needs only wk + the first 128 T columns.
            nc.sync.dma_start(qa0[:, 0:4, :], qT_r[:, 0:4, 0:fc_w])
            nc.sync.dma_start(wq_sb[:, 0:4], wq_r[:, 0:4])
            nc.sync.dma_start(qa0[:, 4:8, :], qT_r[:, 4:8, 0:fc_w])
            nc.sync.dma_start(wq_sb[:, 4:8], wq_r[:, 4:8])
            nc.sync.dma_start(wk_sb[:], wk_r)
            nc.sync.dma_start(sa0[:, :, 0:P], sT_r[:, :, 0:P])
            for a in range(2):
                q_piece(0, a, 0, vaccp)
                q_piece(0, a, 1, vaccp)
            k_proj(0, vaccp, slice(0, P))
            produce(0, 0, mul=False)
            nc.sync.dma_start(sa0[:, :, P:fc_w], sT_r[:, :, P:fc_w])
            nc.sync.dma_start(wv_sb[:], wv_d.ap().rearrange("(ko p) m -> p ko m", p=P))
            nc.sync.dma_start(wo_sb[:], wo_d.ap().rearrange("(ko p) m -> p ko m", p=P))
            nc.sync.dma_start(ident[:], id_d.ap())
            k_proj(0, vaccp, slice(P, fc_w))
            next_v = 0
            for tt in range(1, FPC):
                produce(0, tt, mul=False)
                v_piece(next_v, vaccp); next_v += 1
            # chunk-0 bias muls (DVE-only) are deferred so they never sit in
            # front of the k/v copies the next score tiles depend on.
            for sc in range(1, T // fc_w):
                sa = sap.tile([P, KC, fc_w], bf16, tag="sa", name="sa")
                nc.sync.dma_start(sa[:], sT_r[:, :, sc * fc_w:(sc + 1) * fc_w])
                sa_tiles[sc] = sa
                for tl in range(FPC):
                    tt = sc * FPC + tl
                    if tl == 0:
                        k_proj(sc, vaccp, slice(0, P))
                        produce(0, tt, mul=False)
                        k_proj(sc, vaccp, slice(P, fc_w))
                    else:
                        produce(0, tt, mul=False)
                    if tl == 1:
                        for mt in range(4 * (sc - 1), 4 * sc):
                            produce_mul(0, mt, dve_only=True)
                    elif tl >= 2 and next_v < 8:
                        v_piece(next_v, vaccp); next_v += 1
            q_dma(1)
            for a in range(2):
                q_piece(1, a, 0, vaccp)
                q_piece(1, a, 1, vaccp)
            for mt in range(12, NTT):
                produce_mul(0, mt, dve_only=True)

            # Iteration 1: produce chunk 1, consume chunk 0 (1/slot);
            # late v pieces and q2 quarters in the slack.
            alloc_acc(0)
            for tt in range(NTT):
                produce(1, tt)
                if tt <= 7:
                    v_piece(tt + 8, finp)
                if tt == 8:
                    q_dma(2)
                if tt in (8, 10, 12, 14):
                    q_piece(2, (tt - 8) // 4, ((tt - 8) // 2) % 2, finp)
                consume(0, tt)
            fin_normalize(0)

            # Iteration 2: produce 2, consume 1 (1/slot); finish(0) pieces
            # + q3 quarters in the slack.
            alloc_acc(1)
            for tt in range(NTT):
                produce(2, tt)
                if tt in (1, 2):
                    fin_tr(0, tt - 1, finp)
                elif 3 <= tt <= 10:
                    fin_pso(0, tt - 3, finp)
                if tt == 11:
                    q_dma(3)
                if 11 <= tt <= 14:
                    q_piece(3, (tt - 11) // 2, (tt - 11) % 2, finp)
                consume(1, tt)
            fin_normalize(1)

            # Iteration 3: produce 3; consume 2 compressed 2/slot so chunk 3
            # can start consuming mid-iteration; finish(1) then finish(2)
            # pieces in the slack.
            alloc_acc(2)
            c2 = 0
            c3 = 0
            for tt in range(NTT):
                produce(3, tt)
                if tt in (1, 2):
                    fin_tr(1, tt - 1, finp)
                elif 3 <= tt <= 10:
                    fin_pso(1, tt - 3, finp)
                if tt <= 7:
                    consume(2, c2); consume(2, c2 + 1); c2 += 2
                elif tt == 8:
                    fin_normalize(2)
                    alloc_acc(3)
                    consume(3, c3); c3 += 1
                elif tt in (9, 10):
                    fin_tr(2, tt - 9, finp)
                    consume(3, c3); c3 += 1
                else:
                    fin_pso(2, tt - 11, finp)
                    consume(3, c3); consume(3, c3 + 1); c3 += 2
            # Tail: remaining chunk-3 consumes, last finishes. finish(3)
            # runs through the (now free) score banks, finish(2) leftovers
            # through the finish bank concurrently.
            while c3 < NTT:
                consume(3, c3); c3 += 1
            fin_pso(2, 5, finp)
            fin_pso(2, 6, finp)
            fin_pso(2, 7, finp)
            fin_normalize(3)
            # last chunk: per-F-tile pipeline (transpose -> attnT -> outproj
            # -> staging copy alternating ACT/DVE -> DMA) through the freed
            # score banks.
            flat3 = flats.pop(3)
            for fl in range(FPC):
                ft = (NFC - 1) * FPC + fl
                fcols = slice((NFC - 1) * fc_w + fl * P, (NFC - 1) * fc_w + (fl + 1) * P)
                trS = psS.tile([P, 2, 512], f32, tag="st", name="trS")
                for m in range(2):
                    nc.tensor.transpose(
                        trS[:, m, 0:P],
                        flat3[:, fl, 2 * m:2 * m + 2, :], ident[:])
                    nc.vector.tensor_copy(
                        attnT_sb[:, m, fcols], trS[:, m, 0:P])
                psoS = psS.tile([P, 2, 512], f32, tag="st", name="psoS")
                for cc in range(2):
                    for m in range(2):
                        nc.tensor.matmul(
                            psoS[:, cc, :],
                            lhsT=attnT_sb[:, m, ft * P:(ft + 1) * P],
                            rhs=wo_sb[:, m, cc * 512:(cc + 1) * 512],
                            start=(m == 0), stop=(m == 1))
                ot2 = outp.tile([P, 2, 512], f32, tag="o2", bufs=2, name="ot2")
                if fl % 2 == 0:
                    nc.scalar.copy(ot2[:], psoS[:])
                else:
                    nc.vector.tensor_copy(ot2[:], psoS[:])
                nc.sync.dma_start(
                    out_d.ap()[ft * P:(ft + 1) * P, :].rearrange(
                        "f (cc c) -> f cc c", cc=2),
                    ot2[:])

    nc.compile()
    return nc


_CACHE = {}


def _get_nc():
    if "nc" not in _CACHE:
        _CACHE["nc"] = build_attention_nc(C=C, F=F, T=T, NHEADS=HEADS // HG, H=DEPTH)
    return _CACHE["nc"]


def kernel(query_input, source_input, bias, wq, wk, wv, wo, **run_kwargs):
    import ml_dtypes
    from concourse.bass_utils import run_bass_kernel_spmd

    bf = ml_dtypes.bfloat16
    q = np.asarray(query_input, dtype=np.float32)
    s = np.asarray(source_input, dtype=np.float32)
    b = np.asarray(bias, dtype=np.float32)
    wq4 = np.asarray(wq, dtype=np.float32) * WSC   # [C, 16, 64]
    wk4 = np.asarray(wk, dtype=np.float32) * WSC
    wv4 = np.asarray(wv, dtype=np.float32) * WSC
    wo4 = np.asarray(wo, dtype=np.float32) / WSC   # [16, 64, C]

    qT = [np.ascontiguousarray(q[i].T).astype(bf) for i in range(B)]
    sT = [np.ascontiguousarray(s[i].T).astype(bf) for i in range(B)]
    ebT = np.exp(np.ascontiguousarray(b[0, 0].T)).astype(bf)
    ident = np.eye(128, dtype=np.float32)

    in_maps = []
    for c in range(N_CORES):
        bi, hg = c // HG, c % HG
        hsl = slice(hg * HG, (hg + 1) * HG)
        # wq/wk: [C, 2(A|B), 4 heads, 32 depth] -> [C, 2, 128]
        wqg = wq4[:, hsl, :].reshape(C, HG, 2, 32).transpose(0, 2, 1, 3)
        wkg = wk4[:, hsl, :].reshape(C, HG, 2, 32).transpose(0, 2, 1, 3)
        in_maps.append({
            "qT": qT[bi],
            "sT": sT[bi],
            "ebT": ebT,
            "wq": np.ascontiguousarray(wqg.reshape(C, 2, 128)).astype(bf),
            "wk": np.ascontiguousarray(wkg.reshape(C, 2, 128)).astype(bf),
            "wv": np.ascontiguousarray(
                wv4[:, hsl, :].reshape(C, HG * DEPTH)).astype(bf),
            "wo": np.ascontiguousarray(
                wo4[hsl, :, :].reshape(HG * DEPTH, C)).astype(bf),
            "ident": ident,
        })

    nc = _get_nc()
    res = run_bass_kernel_spmd(nc, in_maps, core_ids=list(range(N_CORES)), **run_kwargs)
    _CACHE["last_results"] = res

    out = np.empty((B, F, C), np.float32)
    for bi in range(B):
        acc = res.results[bi * HG]["out_p"].astype(np.float32)
        for hg in range(1, HG):
            acc = acc + res.results[bi * HG + hg]["out_p"]
        out[bi] = acc
    return out
